# revision 1
# baseline (speedup 1.0000x reference)
"""Trainium2 Bass kernel for nn_AppearanceComposability (sparse_attention).

Reference semantics, per (b, c) with 64x64 images, 3x3 unfold (pad 1):
  key_uf  = unfold(key)[b]  : [C*9, 4096]   (channel order (C, kh, kw))
  out     = key_uf.view(C, 4096, 9) * query_uf.view(C, 4096, 9)[..., 4:5]
The raw .view interleave means, with K_flat = per-channel flattened patch
block (kk*4096 + l) and similarly Q_flat:
  out_flat[m] = K_flat[m] * qv[m // 9],   qv[i] = Q_flat[9*i + 4]

Implementation, per NeuronCore (one batch of 8), per 128-channel group
(channels on partitions):
  - load key/query 64x64 images into zero-margined SBUF buffers
  - qv built with 9 stride-9 copies from the query buffer (margins supply
    the unfold zero padding) + small strided memsets for the x-edge wraps
  - per kk chunk: one tensor_tensor multiply of the shifted key image with
    a repeat-9 "stretched" qv (step-0 access pattern), group-aligned with
    margin over/underhang; then a stride-64 memset for x-edge columns
  - one [128, 4096] store per chunk (the very last chunk is split into two
    halves so its first half drains during the second half's compute)

Scheduling notes (all profile-driven; see the session's memory file):
all multiplies on DVE only (GpSimd shares an SBUF port with DVE's second
read port — concurrent TTs slow ~2.5x on both); loads chained so exactly
two DMAs share the SDMA engines; qv copies issue ahead of second-group
loads on ACT; eight out slots (two reclaimed from dead q_pad SBUF) with
the final tiles getting dedicated slots so tail TTs never wait on
store-drain recycling.

Data parallel over batch: 8 cores, core b handles batch b. No collectives.
"""
import os
import sys

import numpy as np


def _ensure_path():
    try:
        import concourse  # noqa: F401
    except ImportError:
        for p in ("/opt/trn_rl_repo", "/root/.axon_site/_ro/trn_rl_repo"):
            if os.path.isdir(p):
                sys.path.insert(0, p)
                return


_ensure_path()

import concourse.bacc as bacc  # noqa: E402
import concourse.tile as tile  # noqa: E402
from concourse import mybir  # noqa: E402
from concourse.bass_utils import run_bass_kernel_spmd  # noqa: E402
from concourse.tile import add_dep_helper  # noqa: E402


def _install_ntff_hook_shim():
    """Provide antenv.axon_hooks when the image's antenv lacks it.

    concourse.bass_utils imports it unconditionally on the trace path; the
    boot script degrades silently when it is missing. This shim recreates
    the documented hook using the same ctypes loader the boot script uses.
    """
    try:
        import antenv.axon_hooks  # noqa: F401
        return
    except ImportError:
        pass
    try:
        import types

        import antenv
        holder = {"hook": None, "tried": False}

        def set_axon_ntff_profile_hook(h):
            holder["hook"] = h
            holder["tried"] = True

        def get_axon_ntff_profile_hook():
            if not holder["tried"]:
                holder["tried"] = True
                try:
                    from trn_agent_boot.trn_boot import _ntff_profile_via_ctypes
                    so = "/opt/axon/libaxon_pjrt.so"
                    if os.path.exists(so):
                        holder["hook"] = _ntff_profile_via_ctypes(so)
                except Exception:
                    holder["hook"] = None
            return holder["hook"]

        mod = types.ModuleType("antenv.axon_hooks")
        mod.set_axon_ntff_profile_hook = set_axon_ntff_profile_hook
        mod.get_axon_ntff_profile_hook = get_axon_ntff_profile_hook
        sys.modules["antenv.axon_hooks"] = mod
        antenv.axon_hooks = mod
    except Exception:
        pass


_install_ntff_hook_shim()

F32 = mybir.dt.float32

B = 8          # batch == number of cores
C = 256        # channels
H = W = 64
L = H * W      # 4096 pixels
K2 = 9         # 3x3 patch
M = L * K2     # 36864 per-channel output length
MARG = 80      # input image margin (>= 73 needed)
OM = 8         # output tile margin (>= 8 needed)
OFFS = [(kh - 1) * W + (kw - 1) for kh in range(3) for kw in range(3)]

# All multiplies run on DVE. GpSimd tensor_tensor shares an SBUF port pair
# with DVE's second read port; concurrent DVE-TT + GpSimd-TT measured ~2.5x
# slower on both engines (exclusive port lock), so offloading to GpSimd is
# a net loss. DVE alone (~81 us) stays below the DMA floor (~110 us).
POOL_KKS = ()


def _ceil_div(a, b):
    return -(-a // b)


def _plan_qv_ops():
    """Per kk: (i_lo, i_hi, src_start, memsets) for qv[i] = Q_flat[9i+4]."""
    ops = []
    for kk in range(K2):
        s = L * kk
        i_lo = max(0, _ceil_div(s - 4, 9))
        i_hi = min(L, _ceil_div(s + L - 4, 9))
        src_start = 9 * i_lo + 4 - s + OFFS[kk]
        memsets = []
        kw = kk % 3
        if kw != 1:
            target = 0 if kw == 0 else 63
            i0 = (57 * (target - 4 + s)) % 64  # 57 = 9^-1 mod 64
            first = i_lo + ((i0 - i_lo) % 64)
            if first < i_hi:
                cnt = (i_hi - 1 - first) // 64 + 1
                memsets.append((first, cnt, 64))
        ops.append((i_lo, i_hi, src_start, memsets))
    return ops


def _plan_tt_ops():
    """Per kk: (g_lo, g_hi, ngroups, q0); TT covers l in [g_lo, g_hi)."""
    ops = []
    for kk in range(K2):
        s = L * kk
        g_lo = -(s % 9)
        g_hi = L + ((-(s + L)) % 9)
        ops.append((g_lo, g_hi, (g_hi - g_lo) // 9, (s + g_lo) // 9))
    return ops


QV_OPS = _plan_qv_ops()
TT_OPS = _plan_tt_ops()


# Store-merge plan: adjacent chunks in the same kh row can share one tile
# and one (2x bigger) store; the boundary group is patched by a tiny TT with
# a broadcast qv scalar (the first chunk's aligned overhang into the second
# chunk's head region is exactly the region the tiny TT rewrites).
MERGE_PLAN = [(0,), (1, 2), (3, 4), (5,), (6,), (7, 8)]
SINGLE_PLAN = [(kk,) for kk in range(K2)]


def build_graph(use_outs2: bool = True, dve_head: int = 2,
                merge_pairs: bool = False, split_tail: bool = True,
                split_head: bool = True):
    nc = bacc.Bacc(None, target_bir_lowering=False)
    key_ext = nc.declare_dram_parameter("key_map", [C, L], F32, isOutput=False)
    query_ext = nc.declare_dram_parameter("query_map", [C, L], F32, isOutput=False)
    out_ext = nc.declare_dram_parameter("out", [C, M], F32, isOutput=True)

    ngroups = C // 128
    with tile.TileContext(nc) as tc:
        with (
            tc.tile_pool(name="pads", bufs=1) as pads,
            tc.tile_pool(name="qvp", bufs=1) as qvp,
            tc.tile_pool(name="outs", bufs=6) as outs,
        ):
            key_pads, q_pads, qvs = [], [], []
            # Phase 0: prefetch inputs and build both groups' qv on ACT
            # before any store is queued. Ring discipline learned from
            # profiles: (a) concurrent DMAs split the 16 SDMA engines at
            # packet granularity — exactly two in flight (one per HWDGE
            # ring) gives ~420 GB/s aggregate, one alone only ~240; (b) the
            # second group's load *issues* are placed after the first
            # group's ACT copies so the ACT sequencer is never blocked on a
            # load semaphore ahead of the copies; (c) the first two qv
            # copies run on the (otherwise idle) DVE so TT0 starts sooner.
            # q_pads live in their own pool, closed after qv is built, so
            # its SBUF space can be reused for two extra late out slots.
            padq_ctx = tc.tile_pool(name="padq", bufs=1)
            padq = padq_ctx.__enter__()
            for g in range(ngroups):
                q_pad = padq.tile([128, MARG + L + MARG], F32,
                                  name=f"q_pad{g}", tag=f"q_pad{g}")
                nc.vector.memset(q_pad[:, 0:MARG], 0.0)
                nc.vector.memset(q_pad[:, MARG + L:MARG + L + MARG], 0.0)
                key_pad = pads.tile([128, MARG + L + MARG], F32,
                                    name=f"key_pad{g}", tag=f"key_pad{g}")
                nc.vector.memset(key_pad[:, 0:MARG], 0.0)
                nc.vector.memset(key_pad[:, MARG + L:MARG + L + MARG], 0.0)
                key_pads.append(key_pad)
                q_pads.append(q_pad)

            if split_head:
                # First loads in two chained half-pairs: pair1 (q0a||k0a,
                # 2 MB total) lands ~5us before a full 4 MB pair would, so
                # the first half-chunk multiply starts that much earlier.
                hL = L // 2
                q0a = nc.sync.dma_start(q_pads[0][:, MARG:MARG + hL],
                                        query_ext[0:128, 0:hL])
                k0a = nc.scalar.dma_start(key_pads[0][:, MARG:MARG + hL],
                                          key_ext[0:128, 0:hL])
                q0dma = nc.sync.dma_start(q_pads[0][:, MARG + hL:MARG + L],
                                          query_ext[0:128, hL:L])
                add_dep_helper(q0dma.ins, q0a.ins, sync=True,
                               reason="chain load half-pairs")
                k0dma = nc.scalar.dma_start(key_pads[0][:, MARG + hL:MARG + L],
                                            key_ext[0:128, hL:L])
                add_dep_helper(k0dma.ins, k0a.ins, sync=True,
                               reason="chain load half-pairs")
            else:
                q0dma = nc.sync.dma_start(q_pads[0][:, MARG:MARG + L],
                                          query_ext[0:128, :])
                k0dma = nc.scalar.dma_start(key_pads[0][:, MARG:MARG + L],
                                            key_ext[0:128, :])

            def emit_qv(g, dve_head=0, split0_at=None):
                # dve_head: run the first N copies on DVE (idle before TT0)
                # so TT0's qv inputs are ready sooner than ACT can serve
                # them; ACT fills in the rest concurrently. split0_at: emit
                # copy 0 as two halves so the first only needs the first
                # half-load of the query image.
                qv = qvp.tile([128, L], F32, name=f"qv{g}", tag=f"qv{g}")

                def one_copy(kk, a, b):
                    i_lo, _, src_start, _ = QV_OPS[kk]
                    sa = MARG + src_start + 9 * (a - i_lo)
                    dst = qv[:, a:b]
                    src = q_pads[g][:, sa:sa + 9 * (b - a):9]
                    if kk < dve_head:
                        nc.vector.tensor_copy(dst, src)
                    else:
                        nc.scalar.copy(dst, src)

                start = 0
                if split0_at is not None:
                    # Copies for chunks 0 and 1 in half-pieces, a-pieces
                    # first: the a-pieces read only the first half-load, so
                    # emitting any b-piece earlier would block the DVE
                    # sequencer on the second half-load.
                    splits = {0: QV_OPS[0][0] + split0_at,
                              1: QV_OPS[1][0] + split0_at}
                    for kk in (0, 1):
                        one_copy(kk, QV_OPS[kk][0], splits[kk])
                    for kk in (0, 1):
                        one_copy(kk, splits[kk], QV_OPS[kk][1])
                    start = 2
                for kk in range(start, K2):
                    one_copy(kk, QV_OPS[kk][0], QV_OPS[kk][1])
                qvs.append(qv)

            emit_qv(0, dve_head=dve_head,
                    split0_at=228 if split_head else None)

            q1dma = nc.sync.dma_start(q_pads[1][:, MARG:MARG + L],
                                      query_ext[128:256, :])
            add_dep_helper(q1dma.ins, q0dma.ins, sync=True,
                           reason="serialize group loads on ring")
            k1dma = nc.scalar.dma_start(key_pads[1][:, MARG:MARG + L],
                                        key_ext[128:256, :])
            add_dep_helper(k1dma.ins, k0dma.ins, sync=True,
                           reason="serialize group loads on ring")

            emit_qv(1)

            # q_pads fully consumed; release their SBUF for late out slots.
            outs2_ctx = outs2 = None
            if use_outs2:
                padq_ctx.__exit__(None, None, None)
                outs2_ctx = tc.tile_pool(name="outs2", bufs=1)
                outs2 = outs2_ctx.__enter__()

            plan = MERGE_PLAN if merge_pairs else SINGLE_PLAN
            # Phase 1: per group — the chunk multiplies. The qv edge memset
            # for chunk kk is emitted just before TT kk (TT kk's qv read
            # range only ever overlaps chunk kk's own memset), so DVE never
            # stalls waiting for late ACT copies.
            prev_colmset = None
            si = 0
            for g in range(ngroups):
                rows = slice(g * 128, (g + 1) * 128)
                key_pad, qv = key_pads[g], qvs[g]

                for chunks in plan:
                    if (split_head and not merge_pairs
                            and g == 0 and chunks == (1,)):
                        continue  # emitted interleaved with chunk 0
                    width = len(chunks) * L
                    pair = len(chunks) > 1
                    # Late tiles cycle through the slot(s) reclaimed from
                    # q_pad space (safe: q_pad reads long finished by then).
                    if merge_pairs:
                        late = (outs2 is not None and chunks[0] == 7)
                        opool = outs2 if late else outs
                        otag = "otL" if late else ("otp" if pair else "ot1")
                        obufs = 1 if late else 2
                    else:
                        # Only the final two tiles use the reclaimed slots —
                        # one slot each, so the tail TTs never wait on a
                        # store-drain recycle (a 2-slot rotation over the
                        # last four tiles stalled DVE ~13us before the last
                        # TT when the store backlog was deep).
                        late = (outs2 is not None
                                and g == ngroups - 1 and chunks[0] >= 7)
                        opool = outs2 if late else outs
                        otag = "ot2" if late else "ot"
                        obufs = 2 if late else 6
                    ot = opool.tile([128, OM + width + OM], F32,
                                    name=f"ot{g}_{chunks[0]}", tag=otag,
                                    bufs=obufs)

                    if (split_head and not merge_pairs
                            and g == 0 and chunks == (0,)):
                        # Chunks 0 AND 1 in interleaved halves: the a-halves
                        # (l < 2052) need only the first half-loads, so DVE
                        # runs TT0a, TT1a before the full images land; the
                        # b-halves follow once the second half-pair arrives.
                        # Chunk 1 (kw==1) needs no memsets, which is what
                        # makes it the cheap gap-filler.
                        ot1 = outs.tile([128, OM + L + OM], F32,
                                        name="ot0_1", tag="ot", bufs=6)
                        nga = 228
                        qm = QV_OPS[0][3][0]  # (28, 7, 64)
                        cnt_a = sum(1 for j in range(qm[1])
                                    if qm[0] + j * qm[2] < nga)
                        qmsets = [(qm[0], cnt_a),
                                  (qm[0] + cnt_a * qm[2], qm[1] - cnt_a)]
                        prev_tt = None

                        def half_tt(kk, dest, half):
                            nonlocal prev_tt
                            g_lo, g_hi, ng, q00 = TT_OPS[kk]
                            if half == 0:
                                lo, n_g, qs = g_lo, nga, q00
                            else:
                                lo = g_lo + 9 * nga
                                n_g, qs = ng - nga, q00 + nga
                            hi = lo + 9 * n_g
                            dst = dest[:, OM + lo:OM + hi].rearrange(
                                "p (n k) -> p n k", k=9)
                            src_k = key_pad[:, MARG + lo + OFFS[kk]:
                                            MARG + hi + OFFS[kk]].rearrange(
                                "p (n k) -> p n k", k=9)
                            src_q = qv[:, qs:qs + n_g].unsqueeze(
                                2).broadcast_to([128, n_g, 9])
                            tt = nc.vector.tensor_mul(dst, src_k, src_q)
                            if prev_tt is not None:
                                add_dep_helper(tt.ins, prev_tt.ins,
                                               sync=False,
                                               reason="head interleave order")
                            prev_tt = tt

                        for half in (0, 1):
                            mf, mc = qmsets[half]
                            qms = nc.vector.memset(
                                qv[:, mf:mf + (mc - 1) * 64 + 1:64], 0.0)
                            if prev_tt is not None:
                                add_dep_helper(qms.ins, prev_tt.ins,
                                               sync=False,
                                               reason="head interleave order")
                            prev_tt = qms
                            half_tt(0, ot, half)
                            colm = nc.vector.memset(
                                ot[:, OM + (0 if half == 0 else 2112):
                                   OM + (9 * nga if half == 0 else L):64],
                                0.0)
                            add_dep_helper(colm.ins, prev_tt.ins, sync=False,
                                           reason="head interleave order")
                            prev_tt = colm
                            half_tt(1, ot1, half)
                        prev_colmset = None
                        nc.sync.dma_start(out_ext[rows, 0:L],
                                          ot[:, OM:OM + L])
                        nc.scalar.dma_start(out_ext[rows, L:2 * L],
                                            ot1[:, OM:OM + L])
                        si += 2
                        continue

                    if (split_tail and not merge_pairs
                            and g == ngroups - 1 and chunks == (8,)):
                        # Final chunk: split TT + store into halves so the
                        # first half's store drains while the second half
                        # computes — the tail is the one place store latency
                        # isn't hidden by backlog (slow device states).
                        kk = 8
                        for (first, cnt, stride) in QV_OPS[kk][3]:
                            nc.vector.memset(
                                qv[:, first:
                                   first + (cnt - 1) * stride + 1:stride],
                                0.0)
                        g_lo, g_hi, ng, q0 = TT_OPS[kk]
                        nga = ng // 2
                        mid = g_lo + 9 * nga
                        pieces = [(g_lo, nga, q0, 0, mid, 63),
                                  (mid, ng - nga, q0 + nga, mid, L, 2047)]
                        for (lo, n_g, qs, s_lo, s_hi, col0) in pieces:
                            hi = lo + 9 * n_g
                            dst = ot[:, OM + lo:OM + hi].rearrange(
                                "p (n k) -> p n k", k=9)
                            src_k = key_pad[:, MARG + lo + OFFS[kk]:
                                            MARG + hi + OFFS[kk]].rearrange(
                                "p (n k) -> p n k", k=9)
                            src_q = qv[:, qs:qs + n_g].unsqueeze(
                                2).broadcast_to([128, n_g, 9])
                            tt = nc.vector.tensor_mul(dst, src_k, src_q)
                            if prev_colmset is not None:
                                add_dep_helper(
                                    tt.ins, prev_colmset.ins, sync=False,
                                    reason="colmset before next TT")
                            prev_colmset = nc.vector.memset(
                                ot[:, OM + col0:OM + s_hi:64], 0.0)
                            deng = nc.sync if si % 2 == 0 else nc.scalar
                            si += 1
                            deng.dma_start(
                                out_ext[rows, kk * L + s_lo:kk * L + s_hi],
                                ot[:, OM + s_lo:OM + s_hi])
                        continue

                    for idx, kk in enumerate(chunks):
                        for (first, cnt, stride) in QV_OPS[kk][3]:
                            nc.vector.memset(
                                qv[:, first:
                                   first + (cnt - 1) * stride + 1:stride],
                                0.0)
                        g_lo, g_hi, ng, q0 = TT_OPS[kk]
                        base = OM + idx * L
                        if idx > 0:
                            # Boundary group: first (9 - kk) outputs of this
                            # chunk share qv[455*kk]; the previous chunk's
                            # aligned overhang wrote garbage here.
                            p = 9 - kk
                            nc.vector.tensor_mul(
                                ot[:, base:base + p],
                                key_pad[:, MARG + OFFS[kk]:
                                        MARG + OFFS[kk] + p],
                                qv[:, q0:q0 + 1].broadcast_to([128, p]))
                            g_lo, q0, ng = p, q0 + 1, ng - 1
                        dst = ot[:, base + g_lo:base + g_hi].rearrange(
                            "p (n k) -> p n k", k=9)
                        src_k = key_pad[:, MARG + g_lo + OFFS[kk]:
                                        MARG + g_hi + OFFS[kk]].rearrange(
                            "p (n k) -> p n k", k=9)
                        src_q = qv[:, q0:q0 + ng].unsqueeze(2).broadcast_to(
                            [128, ng, 9])
                        eng = nc.gpsimd if kk in POOL_KKS else nc.vector
                        tt = eng.tensor_mul(dst, src_k, src_q)
                        if prev_colmset is not None:
                            # Pin DVE order TT_k -> colmset_k -> TT_{k+1}:
                            # the scheduler otherwise runs the next TT first,
                            # holding the finished tile's store back ~4.4us.
                            add_dep_helper(tt.ins, prev_colmset.ins,
                                           sync=False,
                                           reason="colmset before next TT")
                            prev_colmset = None

                        kw = kk % 3
                        if kw == 0:
                            prev_colmset = nc.vector.memset(
                                ot[:, base:base + L:64], 0.0)
                        elif kw == 2:
                            prev_colmset = nc.vector.memset(
                                ot[:, base + 63:base + L:64], 0.0)

                    deng = nc.sync if si % 2 == 0 else nc.scalar
                    si += 1
                    deng.dma_start(
                        out_ext[rows,
                                chunks[0] * L:(chunks[-1] + 1) * L],
                        ot[:, OM:OM + width])
            if outs2_ctx is not None:
                outs2_ctx.__exit__(None, None, None)
            else:
                padq_ctx.__exit__(None, None, None)
    nc.compile()
    return nc


_GRAPH_CACHE = {}


def _get_graph():
    if "nc" not in _GRAPH_CACHE:
        _GRAPH_CACHE["nc"] = build_graph()
    return _GRAPH_CACHE["nc"]


def kernel(key_map: np.ndarray, query_map: np.ndarray,
           _trace: bool = False, _tmpdir: str | None = None):
    key_map = np.ascontiguousarray(key_map, dtype=np.float32)
    query_map = np.ascontiguousarray(query_map, dtype=np.float32)
    assert key_map.shape == (B, C, H, W), key_map.shape

    nc = _get_graph()
    in_maps = [
        {"key_map": key_map[b].reshape(C, L),
         "query_map": query_map[b].reshape(C, L)}
        for b in range(B)
    ]
    res = run_bass_kernel_spmd(
        nc, in_maps, core_ids=list(range(B)),
        trace=_trace, tmpdir=_tmpdir,
    )
    out = np.stack([res.results[b]["out"] for b in range(B)])
    _GRAPH_CACHE["last_exec_time_ns"] = res.exec_time_ns
    _GRAPH_CACHE["last_results"] = res
    return out.reshape(B, C, L, K2)



# revision 4
# speedup vs baseline: 1.2297x; 1.2297x over previous
"""Trainium2 Bass kernel for nn_AppearanceComposability (sparse_attention).

Reference semantics, per (b, c) with 64x64 images, 3x3 unfold (pad 1):
  out_flat[m] = K_flat[m] * qv[m // 9],   qv[i] = Q_flat[9*i + 4]
where K_flat / Q_flat are the per-channel flattened unfold blocks
(kk*4096 + l, channel order (C, kh, kw)).

v2 implementation (bf16 end-to-end; rel err ~2.9e-3 vs gate 2e-2):
  - all tensors bf16: halves both DMA traffic and enables DVE 2x packing
  - chunks are processed in merged PAIR units (16KB DRAM row segments per
    store, vs 8KB singles: ~25% better effective DMA rate)
  - most chunks are "stretched": ACT pre-builds qs[l] = qv[(s+l)//9] by
    broadcast-copy straight out of the padded query image (runs at ~1
    elem/cyc; the 9x fan-out amortizes the strided read), then DVE does a
    flat contiguous bf16 tensor_tensor multiply which packs 2 elem/cyc
  - a few chunks stay "broadcast" on DVE (classic step-0 qv operand, 1
    elem/cyc) to balance the two engines' load
  - query-wrap zeros: for stretched chunks, 9-wide runs zeroed on the qs
    tile (DVE memset); for broadcast chunks, qv edge memsets as before
  - key-wrap zeros (x-edge columns): stride-64 memsets on the out tile
  - data parallel over batch: 8 cores, core b handles batch b
"""
import os
import sys

import numpy as np


def _ensure_path():
    try:
        import concourse  # noqa: F401
    except ImportError:
        for p in ("/opt/trn_rl_repo", "/root/.axon_site/_ro/trn_rl_repo"):
            if os.path.isdir(p):
                sys.path.insert(0, p)
                return


_ensure_path()

import concourse.bacc as bacc  # noqa: E402
import concourse.tile as tile  # noqa: E402
from concourse import mybir  # noqa: E402
from concourse.bass_utils import run_bass_kernel_spmd  # noqa: E402
from concourse.tile import add_dep_helper  # noqa: E402


def _install_ntff_hook_shim():
    """Provide antenv.axon_hooks when the image's antenv lacks it."""
    try:
        import antenv.axon_hooks  # noqa: F401
        return
    except ImportError:
        pass
    try:
        import types

        import antenv
        holder = {"hook": None, "tried": False}

        def set_axon_ntff_profile_hook(h):
            holder["hook"] = h
            holder["tried"] = True

        def get_axon_ntff_profile_hook():
            if not holder["tried"]:
                holder["tried"] = True
                try:
                    from trn_agent_boot.trn_boot import _ntff_profile_via_ctypes
                    so = "/opt/axon/libaxon_pjrt.so"
                    if os.path.exists(so):
                        holder["hook"] = _ntff_profile_via_ctypes(so)
                except Exception:
                    holder["hook"] = None
            return holder["hook"]

        mod = types.ModuleType("antenv.axon_hooks")
        mod.set_axon_ntff_profile_hook = set_axon_ntff_profile_hook
        mod.get_axon_ntff_profile_hook = get_axon_ntff_profile_hook
        sys.modules["antenv.axon_hooks"] = mod
        antenv.axon_hooks = mod
    except Exception:
        pass


_install_ntff_hook_shim()

F32 = mybir.dt.float32
BF16 = mybir.dt.bfloat16

B = 8          # batch == number of cores
C = 256        # channels
H = W = 64
L = H * W      # 4096 pixels
K2 = 9         # 3x3 patch
M = L * K2     # 36864 per-channel output length
MARG = 80      # input image margin (>= 73 needed)
OM = 8         # output tile margin (>= 8 needed)
QM = 8         # qs tile head margin (stretch group overhang, >= 8)
QTAIL = 580    # qs tile tail pad so run-zero rearrange views stay in-bounds
OFFS = [(kh - 1) * W + (kw - 1) for kh in range(3) for kw in range(3)]


def _ceil_div(a, b):
    return -(-a // b)


def _plan_qv_ops():
    """Per kk: (i_lo, i_hi, src_start, memsets) for qv[i] = Q_flat[9i+4].

    src position (relative to q image start at MARG) of qv[i] is
    src_start + 9*(i - i_lo).  memsets are (first, cnt, 64) runs in
    i-space where the query tap wraps an x-edge (must read as zero).
    """
    ops = []
    for kk in range(K2):
        s = L * kk
        i_lo = max(0, _ceil_div(s - 4, 9))
        i_hi = min(L, _ceil_div(s + L - 4, 9))
        src_start = 9 * i_lo + 4 - s + OFFS[kk]
        memsets = []
        kw = kk % 3
        if kw != 1:
            target = 0 if kw == 0 else 63
            i0 = (57 * (target - 4 + s)) % 64  # 57 = 9^-1 mod 64
            first = i_lo + ((i0 - i_lo) % 64)
            if first < i_hi:
                cnt = (i_hi - 1 - first) // 64 + 1
                memsets.append((first, cnt, 64))
        ops.append((i_lo, i_hi, src_start, memsets))
    return ops


def _plan_tt_ops():
    """Per kk: (g_lo, g_hi, ngroups, q0); TT covers l in [g_lo, g_hi)."""
    ops = []
    for kk in range(K2):
        s = L * kk
        g_lo = -(s % 9)
        g_hi = L + ((-(s + L)) % 9)
        ops.append((g_lo, g_hi, (g_hi - g_lo) // 9, (s + g_lo) // 9))
    return ops


QV_OPS = _plan_qv_ops()
TT_OPS = _plan_tt_ops()

# Units: chunks sharing one SBUF tile and one store (16KB DRAM segments
# per partition row for pairs). Modes per chunk instance: 's' = stretched
# (ACT builds qs, DVE 2x TT), 'b' = broadcast (DVE 1x TT).
UNITS = ((0,), (1, 2), (3, 4), (5, 6), (7, 8))
# (group, kk) pairs processed in broadcast mode — the DVE/ACT balance knob.
BCAST = frozenset({(0, 0), (1, 0), (0, 2), (1, 2)})


def build_graph(bcast=BCAST):
    nc = bacc.Bacc(None, target_bir_lowering=False)
    key_ext = nc.declare_dram_parameter("key_map", [C, L], BF16,
                                        isOutput=False)
    query_ext = nc.declare_dram_parameter("query_map", [C, L], BF16,
                                          isOutput=False)
    out_ext = nc.declare_dram_parameter("out", [C, M], BF16, isOutput=True)

    ngroups = C // 128
    with tile.TileContext(nc) as tc:
        with (
            tc.tile_pool(name="pads", bufs=1) as pads,
            tc.tile_pool(name="qvp", bufs=1) as qvp,
            tc.tile_pool(name="qsp", bufs=3) as qsp,
            tc.tile_pool(name="outs1", bufs=2) as outs1,
            tc.tile_pool(name="outs2", bufs=3) as outs2,
        ):
            key_pads, q_pads, qvs = [], [], []
            for g in range(ngroups):
                q_pad = pads.tile([128, MARG + L + MARG], BF16,
                                  name=f"q_pad{g}", tag=f"q_pad{g}")
                nc.vector.memset(q_pad[:, 0:MARG], 0.0)
                nc.vector.memset(q_pad[:, MARG + L:MARG + L + MARG], 0.0)
                key_pad = pads.tile([128, MARG + L + MARG], BF16,
                                    name=f"key_pad{g}", tag=f"key_pad{g}")
                nc.vector.memset(key_pad[:, 0:MARG], 0.0)
                nc.vector.memset(key_pad[:, MARG + L:MARG + L + MARG], 0.0)
                key_pads.append(key_pad)
                q_pads.append(q_pad)
                qv = qvp.tile([128, L], BF16, name=f"qv{g}", tag=f"qv{g}")
                qvs.append(qv)

            # Loads: two HWDGE rings (sync + scalar queues), groups chained
            # so exactly two DMAs are in flight at a time.
            q0dma = nc.sync.dma_start(q_pads[0][:, MARG:MARG + L],
                                      query_ext[0:128, :])
            k0dma = nc.scalar.dma_start(key_pads[0][:, MARG:MARG + L],
                                        key_ext[0:128, :])
            q1dma = nc.sync.dma_start(q_pads[1][:, MARG:MARG + L],
                                      query_ext[128:256, :])
            add_dep_helper(q1dma.ins, q0dma.ins, sync=True,
                           reason="serialize group loads on ring")
            k1dma = nc.scalar.dma_start(key_pads[1][:, MARG:MARG + L],
                                        key_ext[128:256, :])
            add_dep_helper(k1dma.ins, k0dma.ins, sync=True,
                           reason="serialize group loads on ring")

            def emit_qv_chunk(g, kk):
                """DVE strided copy building qv range for broadcast chunk kk
                (+ its query-wrap edge memsets)."""
                i_lo, i_hi, src_start, msets = QV_OPS[kk]
                dst = qvs[g][:, i_lo:i_hi]
                src = q_pads[g][:, MARG + src_start:
                                MARG + src_start + 9 * (i_hi - i_lo):9]
                nc.vector.tensor_copy(dst, src)
                for (first, cnt, step) in msets:
                    nc.vector.memset(
                        qvs[g][:, first:first + (cnt - 1) * step + 1:step],
                        0.0)

            def emit_stretch(g, qs, s, wu, prev_act):
                """ACT copies building qs[QM+x] = qv[(s+x)//9] for
                x in [0, wu), reading straight out of q_pad. Returns last
                ACT op (for stream-order chaining)."""
                i0 = s // 9
                i1 = (s + wu - 1) // 9
                for kk in range(K2):
                    i_lo, i_hi, src_start, _ = QV_OPS[kk]
                    a, b = max(i_lo, i0), min(i_hi, i1 + 1)
                    if a >= b:
                        continue
                    dst = qs[:, QM + 9 * a - s:QM + 9 * b - s].rearrange(
                        "p (n k) -> p n k", k=9)
                    sa = MARG + src_start + 9 * (a - i_lo)
                    src = q_pads[g][:, sa:sa + 9 * (b - a):9].unsqueeze(
                        2).broadcast_to([128, b - a, 9])
                    op = nc.scalar.copy(dst, src)
                    if prev_act is not None:
                        add_dep_helper(op.ins, prev_act.ins, sync=False,
                                       reason="ACT stream order")
                    prev_act = op
                return prev_act

            def emit_qs_runzeros(qs, s, wu, kks):
                """Zero 9-wide qs runs where the query tap wrapped an
                x-edge (stretched-chunk analogue of the qv edge memsets)."""
                i0 = s // 9
                i1 = (s + wu - 1) // 9
                for kk in kks:
                    i_lo, i_hi, _, msets = QV_OPS[kk]
                    a, b = max(i_lo, i0), min(i_hi, i1 + 1)
                    for (first, cnt, step) in msets:
                        j0 = max(0, _ceil_div(a - first, step))
                        j1 = (b - 1 - first) // step
                        if j0 > j1:
                            continue
                        iz = first + j0 * step
                        cnt2 = j1 - j0 + 1
                        A = QM + 9 * iz - s
                        view = qs[:, A:A + 576 * cnt2].rearrange(
                            "p (n k) -> p n k", k=576)[:, :, 0:9]
                        nc.vector.memset(view, 0.0)

            prev_act = None
            prev_colmset = None
            si = 0
            for g in range(ngroups):
                rows = slice(g * 128, (g + 1) * 128)
                key_pad, qv = key_pads[g], qvs[g]

                # qv builds for this group's broadcast chunks (DVE; cheap)
                for u in UNITS:
                    for kk in u:
                        if (g, kk) in bcast:
                            emit_qv_chunk(g, kk)

                for u in UNITS:
                    wu = len(u) * L
                    s_u = u[0] * L
                    modes = ["b" if (g, kk) in bcast else "s" for kk in u]
                    opool = outs1 if len(u) == 1 else outs2
                    ot = opool.tile([128, OM + wu + OM], BF16,
                                    name=f"ot{g}_{u[0]}",
                                    tag=f"ot{len(u)}")

                    qs = None
                    if "s" in modes:
                        qs = qsp.tile([128, QM + wu + QM + QTAIL], BF16,
                                      name=f"qs{g}_{u[0]}", tag="qs")
                        prev_act = emit_stretch(g, qs, s_u, wu, prev_act)
                        emit_qs_runzeros(
                            qs, s_u, wu,
                            [kk for kk, m in zip(u, modes) if m == "s"])

                    prev_tt = None
                    for idx, (kk, mode) in enumerate(zip(u, modes)):
                        base = OM + idx * L
                        g_lo, g_hi, ng, q0 = TT_OPS[kk]
                        if mode == "s":
                            # flat contiguous bf16 TT -> DVE 2x packing
                            tt = nc.vector.tensor_mul(
                                ot[:, base:base + L],
                                key_pad[:, MARG + OFFS[kk]:
                                        MARG + OFFS[kk] + L],
                                qs[:, QM + idx * L:QM + idx * L + L])
                        else:
                            if idx > 0:
                                # boundary group: first p outputs share
                                # qv[q0]; emit with a tiny broadcast TT
                                p = 9 - kk
                                nc.vector.tensor_mul(
                                    ot[:, base:base + p],
                                    key_pad[:, MARG + OFFS[kk]:
                                            MARG + OFFS[kk] + p],
                                    qv[:, q0:q0 + 1].broadcast_to([128, p]))
                                g_lo, q0, ng = p, q0 + 1, ng - 1
                            dst = ot[:, base + g_lo:base + g_hi].rearrange(
                                "p (n k) -> p n k", k=9)
                            src_k = key_pad[:, MARG + g_lo + OFFS[kk]:
                                            MARG + g_hi + OFFS[kk]].rearrange(
                                "p (n k) -> p n k", k=9)
                            src_q = qv[:, q0:q0 + ng].unsqueeze(
                                2).broadcast_to([128, ng, 9])
                            tt = nc.vector.tensor_mul(dst, src_k, src_q)
                        if prev_tt is not None:
                            add_dep_helper(tt.ins, prev_tt.ins, sync=False,
                                           reason="piece order in tile")
                        if prev_colmset is not None:
                            add_dep_helper(tt.ins, prev_colmset.ins,
                                           sync=False,
                                           reason="colmset before next TT")
                            prev_colmset = None
                        prev_tt = tt

                        kw = kk % 3
                        if kw == 0:
                            prev_colmset = nc.vector.memset(
                                ot[:, base:base + L:64], 0.0)
                            prev_tt = prev_colmset
                        elif kw == 2:
                            prev_colmset = nc.vector.memset(
                                ot[:, base + 63:base + L:64], 0.0)
                            prev_tt = prev_colmset

                    deng = nc.sync if si % 2 == 0 else nc.scalar
                    si += 1
                    deng.dma_start(
                        out_ext[rows, u[0] * L:(u[-1] + 1) * L],
                        ot[:, OM:OM + wu])
    nc.compile()
    return nc


_GRAPH_CACHE = {}


def _get_graph():
    if "nc" not in _GRAPH_CACHE:
        _GRAPH_CACHE["nc"] = build_graph()
    return _GRAPH_CACHE["nc"]


def kernel(key_map: np.ndarray, query_map: np.ndarray,
           _trace: bool = False, _tmpdir: str | None = None):
    import ml_dtypes
    bf16 = ml_dtypes.bfloat16
    key_map = np.ascontiguousarray(key_map, dtype=np.float32).astype(bf16)
    query_map = np.ascontiguousarray(query_map, dtype=np.float32).astype(bf16)
    assert key_map.shape == (B, C, H, W), key_map.shape

    nc = _get_graph()
    in_maps = [
        {"key_map": key_map[b].reshape(C, L),
         "query_map": query_map[b].reshape(C, L)}
        for b in range(B)
    ]
    res = run_bass_kernel_spmd(
        nc, in_maps, core_ids=list(range(B)),
        trace=_trace, tmpdir=_tmpdir,
    )
    out = np.stack([res.results[b]["out"] for b in range(B)])
    _GRAPH_CACHE["last_exec_time_ns"] = res.exec_time_ns
    _GRAPH_CACHE["last_results"] = res
    return out.astype(np.float32).reshape(B, C, L, K2)


# revision 34
# speedup vs baseline: 1.4876x; 1.2097x over previous
"""Trainium2 Bass kernel for nn_AppearanceComposability (sparse_attention).

Reference semantics, per (b, c) with 64x64 images, 3x3 unfold (pad 1):
  out_flat[m] = K_flat[m] * qv[m // 9],   qv[i] = Q_flat[9*i + 4]
where K_flat / Q_flat are the per-channel flattened unfold blocks
(kk*4096 + l, channel order (C, kh, kw)).

v2 implementation (bf16 end-to-end; rel err ~2.9e-3 vs gate 2e-2):
  - all tensors bf16: halves both DMA traffic and enables DVE 2x packing
  - chunks are processed in merged PAIR units (16KB DRAM row segments per
    store, vs 8KB singles: ~25% better effective DMA rate)
  - most chunks are "stretched": ACT pre-builds qs[l] = qv[(s+l)//9] by
    broadcast-copy straight out of the padded query image (runs at ~1
    elem/cyc; the 9x fan-out amortizes the strided read), then DVE does a
    flat contiguous bf16 tensor_tensor multiply which packs 2 elem/cyc
  - a few chunks stay "broadcast" on DVE (classic step-0 qv operand, 1
    elem/cyc) to balance the two engines' load
  - query-wrap zeros: for stretched chunks, 9-wide runs zeroed on the qs
    tile (DVE memset); for broadcast chunks, qv edge memsets as before
  - key-wrap zeros (x-edge columns): stride-64 memsets on the out tile
  - data parallel over batch: 8 cores, core b handles batch b
"""
import os
import sys

import numpy as np


def _ensure_path():
    try:
        import concourse  # noqa: F401
    except ImportError:
        for p in ("/opt/trn_rl_repo", "/root/.axon_site/_ro/trn_rl_repo"):
            if os.path.isdir(p):
                sys.path.insert(0, p)
                return


_ensure_path()

import concourse.bacc as bacc  # noqa: E402
import concourse.tile as tile  # noqa: E402
from concourse import mybir  # noqa: E402
from concourse.bass_utils import run_bass_kernel_spmd  # noqa: E402
from concourse.tile import add_dep_helper  # noqa: E402


def _install_ntff_hook_shim():
    """Provide antenv.axon_hooks when the image's antenv lacks it."""
    try:
        import antenv.axon_hooks  # noqa: F401
        return
    except ImportError:
        pass
    try:
        import types

        import antenv
        holder = {"hook": None, "tried": False}

        def set_axon_ntff_profile_hook(h):
            holder["hook"] = h
            holder["tried"] = True

        def get_axon_ntff_profile_hook():
            if not holder["tried"]:
                holder["tried"] = True
                try:
                    from trn_agent_boot.trn_boot import _ntff_profile_via_ctypes
                    so = "/opt/axon/libaxon_pjrt.so"
                    if os.path.exists(so):
                        holder["hook"] = _ntff_profile_via_ctypes(so)
                except Exception:
                    holder["hook"] = None
            return holder["hook"]

        mod = types.ModuleType("antenv.axon_hooks")
        mod.set_axon_ntff_profile_hook = set_axon_ntff_profile_hook
        mod.get_axon_ntff_profile_hook = get_axon_ntff_profile_hook
        sys.modules["antenv.axon_hooks"] = mod
        antenv.axon_hooks = mod
    except Exception:
        pass


_install_ntff_hook_shim()

F32 = mybir.dt.float32
BF16 = mybir.dt.bfloat16

B = 8          # batch == number of cores
C = 256        # channels
H = W = 64
L = H * W      # 4096 pixels
K2 = 9         # 3x3 patch
M = L * K2     # 36864 per-channel output length
MARG = 80      # input image margin (>= 73 needed)
OM = 8         # output tile margin (>= 8 needed)
QM = 8         # qs tile head margin (stretch group overhang, >= 8)
QTAIL = 580    # qs tile tail pad so run-zero rearrange views stay in-bounds
OFFS = [(kh - 1) * W + (kw - 1) for kh in range(3) for kw in range(3)]


def _ceil_div(a, b):
    return -(-a // b)


def _plan_qv_ops():
    """Per kk: (i_lo, i_hi, src_start, memsets) for qv[i] = Q_flat[9i+4].

    src position (relative to q image start at MARG) of qv[i] is
    src_start + 9*(i - i_lo).  memsets are (first, cnt, 64) runs in
    i-space where the query tap wraps an x-edge (must read as zero).
    """
    ops = []
    for kk in range(K2):
        s = L * kk
        i_lo = max(0, _ceil_div(s - 4, 9))
        i_hi = min(L, _ceil_div(s + L - 4, 9))
        src_start = 9 * i_lo + 4 - s + OFFS[kk]
        memsets = []
        kw = kk % 3
        if kw != 1:
            target = 0 if kw == 0 else 63
            i0 = (57 * (target - 4 + s)) % 64  # 57 = 9^-1 mod 64
            first = i_lo + ((i0 - i_lo) % 64)
            if first < i_hi:
                cnt = (i_hi - 1 - first) // 64 + 1
                memsets.append((first, cnt, 64))
        ops.append((i_lo, i_hi, src_start, memsets))
    return ops


def _plan_tt_ops():
    """Per kk: (g_lo, g_hi, ngroups, q0); TT covers l in [g_lo, g_hi)."""
    ops = []
    for kk in range(K2):
        s = L * kk
        g_lo = -(s % 9)
        g_hi = L + ((-(s + L)) % 9)
        ops.append((g_lo, g_hi, (g_hi - g_lo) // 9, (s + g_lo) // 9))
    return ops


QV_OPS = _plan_qv_ops()
TT_OPS = _plan_tt_ops()

# Units: chunks sharing one SBUF tile and one (two-queue) store. Chunk 0
# (broadcast, ACT-free) sits mid-schedule where its DVE TT fills the gap
# while ACT streams ahead; the schedule then ends on stretched pairs
# whose stores drain promptly. Modes per chunk instance: 's' = stretched
# (ACT builds qs, DVE 2x TT), 'b' = broadcast (DVE 1x TT).
UNITS = ((1, 2), (3, 4), (0,), (5, 6), (7, 8))
# (group, kk) pairs processed in broadcast mode — the DVE/ACT balance knob.
BCAST = frozenset({(0, 0), (1, 0), (0, 2), (1, 2)})
# Where to split the first stretched piece of group 0 (qv group index
# within source chunk 1) so its first stretch+TT only need the first
# half-loads. 9*230 = 2070 output elems.
HEAD_SPLIT = 230
# (group, kk) -> elems: chunk processed stretched up to the cut (which
# must be 9-aligned in global m) and broadcast past it. Fine-grained
# ACT/DVE balance: a half-chunk shifts ~2us of stretch off ACT for
# ~+1us of broadcast on DVE. kk must be the last stretched piece of its
# unit and have kw == 1 (no wrap masks across the cut).
PARTIAL = {(1, 4): 2048}
# Split the terminal piece's TT + store into halves so the drain tail
# overlaps compute (applies to this (group, kk)).
TAIL_SPLIT = (1, 8)


def build_graph(bcast=BCAST):
    nc = bacc.Bacc(None, target_bir_lowering=False)
    key_ext = nc.declare_dram_parameter("key_map", [C, L], BF16,
                                        isOutput=False)
    query_ext = nc.declare_dram_parameter("query_map", [C, L], BF16,
                                          isOutput=False)
    out_ext = nc.declare_dram_parameter("out", [C, M], BF16, isOutput=True)

    ngroups = C // 128
    with tile.TileContext(nc) as tc:
        with (
            tc.tile_pool(name="pads", bufs=1) as pads,
            tc.tile_pool(name="qvp", bufs=1) as qvp,
            tc.tile_pool(name="qsp", bufs=4) as qsp,
            tc.tile_pool(name="outs1", bufs=2) as outs1,
            tc.tile_pool(name="outs2", bufs=3) as outs2,
        ):
            key_pads, q_pads, qvs = [], [], []
            for g in range(ngroups):
                q_pad = pads.tile([128, MARG + L + MARG], BF16,
                                  name=f"q_pad{g}", tag=f"q_pad{g}")
                nc.vector.memset(q_pad[:, 0:MARG], 0.0)
                nc.vector.memset(q_pad[:, MARG + L:MARG + L + MARG], 0.0)
                key_pad = pads.tile([128, MARG + L + MARG], BF16,
                                    name=f"key_pad{g}", tag=f"key_pad{g}")
                nc.vector.memset(key_pad[:, 0:MARG], 0.0)
                nc.vector.memset(key_pad[:, MARG + L:MARG + L + MARG], 0.0)
                key_pads.append(key_pad)
                q_pads.append(q_pad)
                qv = qvp.tile([128, L], BF16, name=f"qv{g}", tag=f"qv{g}")
                qvs.append(qv)

            # Loads: all but one ride the sync queue (the sync sequencer
            # has nothing better to do than wait on the chain). The very
            # first key half goes on the scalar queue so it lands in
            # parallel with the first query half — ACT is idle then, so
            # the one inline wait costs nothing. Keeping the scalar queue
            # otherwise clear of loads is crucial: a chained load issue
            # sitting in the ACT instruction stream blocks the stretch
            # pipeline on load-completion semaphores.
            hL = L // 2
            nc.scalar.dma_start(key_pads[0][:, MARG:MARG + hL],
                                key_ext[0:128, 0:hL])
            # Chained so each load completes before the next starts:
            # unchained, all seven transfer concurrently and the FIRST
            # half arrives ~5x later, starving the early pipeline (and
            # the idle engines then also run at lower clocks).
            seq = [(0, "q", 0), (0, "q", 1), (0, "k", 1),
                   (1, "q", 0), (1, "k", 0), (1, "q", 1), (1, "k", 1)]
            prev_q = None
            for (g, t, h) in seq:
                pad = q_pads[g] if t == "q" else key_pads[g]
                ext = query_ext if t == "q" else key_ext
                qd = nc.sync.dma_start(
                    pad[:, MARG + h * hL:MARG + (h + 1) * hL],
                    ext[g * 128:(g + 1) * 128, h * hL:(h + 1) * hL])
                if prev_q is not None:
                    add_dep_helper(qd.ins, prev_q.ins, sync=True,
                                   reason="chain loads on sync queue")
                prev_q = qd

            def emit_qv_chunk(g, kk, j_lo=None):
                """DVE strided copies building the qv range broadcast chunk
                kk's TT reads: [s//9, ceil((s+L)/9)-1], which straddles into
                neighbor source chunks (+ query-wrap edge memsets, clipped
                per segment). j_lo override: partial chunks only need the
                post-cut range."""
                s = kk * L
                if j_lo is None:
                    j_lo = s // 9
                j_hi = _ceil_div(s + L, 9) - 1
                for kk2 in range(max(0, kk - 1), min(K2, kk + 2)):
                    i_lo, i_hi, src_start, msets = QV_OPS[kk2]
                    a, b = max(i_lo, j_lo), min(i_hi, j_hi + 1)
                    if a >= b:
                        continue
                    dst = qvs[g][:, a:b]
                    sa = MARG + src_start + 9 * (a - i_lo)
                    src = q_pads[g][:, sa:sa + 9 * (b - a):9]
                    nc.vector.tensor_copy(dst, src)
                    for (first, cnt, step) in msets:
                        j0 = max(0, _ceil_div(a - first, step))
                        j1 = (b - 1 - first) // step
                        if j0 > j1:
                            continue
                        f2 = first + j0 * step
                        c2 = j1 - j0 + 1
                        nc.vector.memset(
                            qvs[g][:, f2:f2 + (c2 - 1) * step + 1:step],
                            0.0)

            def emit_stretch(g, qs, s, wu, prev_act, splits=()):
                """ACT copies building qs[QM+x] = qv[(s+x)//9] for
                x in [0, wu), reading straight out of q_pad (the 9x fan-out
                amortizes the strided read; measured ~1.08 ns/elem).
                splits: qv group indices at which to break a copy so the
                early piece only depends on the first half-load. Returns
                the last ACT op (for stream-order chaining)."""
                i0 = s // 9
                i1 = (s + wu - 1) // 9
                for kk in range(K2):
                    i_lo, i_hi, src_start, _ = QV_OPS[kk]
                    a, b = max(i_lo, i0), min(i_hi, i1 + 1)
                    if a >= b:
                        continue
                    cuts = [a] + [c for c in splits if a < c < b] + [b]
                    for a2, b2 in zip(cuts[:-1], cuts[1:]):
                        dst = qs[:, QM + 9 * a2 - s:
                                 QM + 9 * b2 - s].rearrange(
                            "p (n k) -> p n k", k=9)
                        sa = MARG + src_start + 9 * (a2 - i_lo)
                        src = q_pads[g][:, sa:sa + 9 * (b2 - a2):9].unsqueeze(
                            2).broadcast_to([128, b2 - a2, 9])
                        op = nc.scalar.copy(dst, src)
                        if prev_act is not None:
                            add_dep_helper(op.ins, prev_act.ins, sync=False,
                                           reason="ACT stream order")
                        prev_act = op
                return prev_act

            def emit_qs_runzeros(qs, s, wu, kks, splits=()):
                """Zero 9-wide qs runs where the query tap wrapped an
                x-edge (stretched-chunk analogue of the qv edge memsets).
                splits mirror emit_stretch's so the early head piece's
                zeros don't depend on the late stretch copies."""
                i0 = s // 9
                i1 = (s + wu - 1) // 9
                for kk in kks:
                    i_lo, i_hi, _, msets = QV_OPS[kk]
                    a, b = max(i_lo, i0), min(i_hi, i1 + 1)
                    if a >= b:
                        continue
                    for (first, cnt, step) in msets:
                        cuts = [a] + [c for c in splits if a < c < b] + [b]
                        for a2, b2 in zip(cuts[:-1], cuts[1:]):
                            j0 = max(0, _ceil_div(a2 - first, step))
                            j1 = (b2 - 1 - first) // step
                            if j0 > j1:
                                continue
                            iz = first + j0 * step
                            cnt2 = j1 - j0 + 1
                            A = QM + 9 * iz - s
                            view = qs[:, A:A + 576 * cnt2].rearrange(
                                "p (n k) -> p n k", k=576)[:, :, 0:9]
                            nc.vector.memset(view, 0.0)

            prev_act = None
            prev_colmset = None
            pending_scalar = None
            qv_built = set()
            for g in range(ngroups):
                rows = slice(g * 128, (g + 1) * 128)
                key_pad, qv = key_pads[g], qvs[g]

                for iu, u in enumerate(UNITS):
                    wu = len(u) * L
                    s_u = u[0] * L
                    modes = ["p" if (g, kk) in PARTIAL else
                             ("b" if (g, kk) in bcast else "s")
                             for kk in u]
                    opool = outs1 if len(u) == 1 else outs2
                    ot = opool.tile([128, OM + wu + OM], BF16,
                                    name=f"ot{g}_{u[0]}",
                                    tag=f"ot{len(u)}")

                    # qs spans only the contiguous run of stretched pieces
                    qs = None
                    if "s" in modes or "p" in modes:
                        sidx = [i for i, m in enumerate(modes)
                                if m in ("s", "p")]
                        f_s, l_s = sidx[0], sidx[-1]
                        s_q = s_u + f_s * L
                        w_q = (l_s - f_s) * L + (
                            PARTIAL[(g, u[l_s])] if modes[l_s] == "p"
                            else L)
                        qs = qsp.tile([128, QM + w_q + QM + QTAIL], BF16,
                                      name=f"qs{g}_{u[0]}", tag="qs")
                        splits = ()
                        if g == 0 and iu == 0:
                            # break the first source at the half-image
                            # boundary (early TT on the first half-load)
                            # and peel the mid-unit straddle group so the
                            # first piece's second TT doesn't wait on the
                            # whole second source's stretch
                            splits = (QV_OPS[u[f_s]][0] + HEAD_SPLIT,
                                      QV_OPS[u[f_s]][1] + 1)
                        prev_act = emit_stretch(g, qs, s_q, w_q, prev_act,
                                                splits)
                        emit_qs_runzeros(
                            qs, s_q, w_q,
                            [kk for kk, m in zip(u, modes)
                             if m in ("s", "p")],
                            splits)

                    # Deferred scalar-queue store half of the PREVIOUS
                    # unit: emitted after this unit's stretch copies so
                    # the ACT sequencer never stalls on it (its data is
                    # long since ready), yet the scalar DMA queue stays
                    # fed in parallel with the sync queue.
                    if pending_scalar is not None:
                        nc.scalar.dma_start(*pending_scalar)
                        pending_scalar = None

                    prev_tt = None
                    for idx, (kk, mode) in enumerate(zip(u, modes)):
                        base = OM + idx * L
                        g_lo, g_hi, ng, q0 = TT_OPS[kk]
                        tail_split = (g, kk) == TAIL_SPLIT
                        if mode in ("s", "p"):
                            # flat contiguous bf16 TT -> DVE 2x packing;
                            # the group-0 head piece is split so TT-a only
                            # needs the first half-loads; the terminal
                            # piece is split so its store drains during
                            # the second half's compute.
                            qb = QM + (idx - f_s) * L
                            send = PARTIAL.get((g, kk), L)
                            pieces = [(0, send)]
                            if g == 0 and iu == 0 and idx == f_s:
                                cut = 9 * HEAD_SPLIT - (9 - kk)
                                pieces = [(0, cut), (cut, send)]
                            elif tail_split:
                                pieces = [(0, send // 2), (send // 2, send)]
                            for (xa, xb) in pieces:
                                tt = nc.vector.tensor_mul(
                                    ot[:, base + xa:base + xb],
                                    key_pad[:, MARG + OFFS[kk] + xa:
                                            MARG + OFFS[kk] + xb],
                                    qs[:, qb + xa:qb + xb])
                                if prev_tt is not None:
                                    add_dep_helper(
                                        tt.ins, prev_tt.ins, sync=False,
                                        reason="piece order in tile")
                                prev_tt = tt
                                if tail_split:
                                    kw = kk % 3
                                    if kw != 1:
                                        c00 = 0 if kw == 0 else 63
                                        first = xa + ((c00 - xa) % 64)
                                        prev_tt = nc.vector.memset(
                                            ot[:, base + first:
                                               base + xb:64], 0.0)
                                    deng = nc.sync if xa == 0 else nc.scalar
                                    deng.dma_start(
                                        out_ext[rows,
                                                u[0] * L + idx * L + xa:
                                                u[0] * L + idx * L + xb],
                                        ot[:, base + xa:base + xb])
                            if mode == "p":
                                # broadcast remainder past the cut (send
                                # is 9-aligned in global m by choice)
                                q0r = (kk * L + send) // 9
                                ngr = (g_hi - send) // 9
                                emit_qv_chunk(g, kk, j_lo=q0r)
                                dst = ot[:, base + send:
                                         base + g_hi].rearrange(
                                    "p (n k) -> p n k", k=9)
                                src_k = key_pad[
                                    :, MARG + send + OFFS[kk]:
                                    MARG + g_hi + OFFS[kk]].rearrange(
                                    "p (n k) -> p n k", k=9)
                                src_q = qv[:, q0r:q0r + ngr].unsqueeze(
                                    2).broadcast_to([128, ngr, 9])
                                tt = nc.vector.tensor_mul(dst, src_k, src_q)
                                add_dep_helper(tt.ins, prev_tt.ins,
                                               sync=False,
                                               reason="piece order in tile")
                                prev_tt = tt
                        else:
                            if (g, kk) not in qv_built:
                                emit_qv_chunk(g, kk)
                                qv_built.add((g, kk))
                            if idx > 0:
                                # boundary group: first p outputs share
                                # qv[q0]; emit with a tiny broadcast TT
                                p = 9 - kk
                                nc.vector.tensor_mul(
                                    ot[:, base:base + p],
                                    key_pad[:, MARG + OFFS[kk]:
                                            MARG + OFFS[kk] + p],
                                    qv[:, q0:q0 + 1].broadcast_to([128, p]))
                                g_lo, q0, ng = p, q0 + 1, ng - 1
                            dst = ot[:, base + g_lo:base + g_hi].rearrange(
                                "p (n k) -> p n k", k=9)
                            src_k = key_pad[:, MARG + g_lo + OFFS[kk]:
                                            MARG + g_hi + OFFS[kk]].rearrange(
                                "p (n k) -> p n k", k=9)
                            src_q = qv[:, q0:q0 + ng].unsqueeze(
                                2).broadcast_to([128, ng, 9])
                            tt = nc.vector.tensor_mul(dst, src_k, src_q)
                            if prev_tt is not None:
                                add_dep_helper(tt.ins, prev_tt.ins,
                                               sync=False,
                                               reason="piece order in tile")
                            prev_tt = tt
                        if prev_colmset is not None:
                            add_dep_helper(tt.ins, prev_colmset.ins,
                                           sync=False,
                                           reason="colmset before next TT")
                            prev_colmset = None

                        kw = kk % 3
                        if not tail_split:
                            if kw == 0:
                                prev_colmset = nc.vector.memset(
                                    ot[:, base:base + L:64], 0.0)
                                prev_tt = prev_colmset
                            elif kw == 2:
                                prev_colmset = nc.vector.memset(
                                    ot[:, base + 63:base + L:64], 0.0)
                                prev_tt = prev_colmset

                        # Store as two halves on BOTH HWDGE queues: one
                        # queue alone tops out ~240 GB/s, two concurrently
                        # ~424. The sync half (= first piece for pairs)
                        # issues as soon as that piece's data is final;
                        # the scalar half is deferred into the next unit's
                        # emission point (see pending_scalar above).
                        hw = wu // 2
                        c0 = u[0] * L
                        if idx == (0 if len(u) > 1 else len(u) - 1):
                            nc.sync.dma_start(out_ext[rows, c0:c0 + hw],
                                              ot[:, OM:OM + hw])
                    if (g, u[-1]) != TAIL_SPLIT:
                        pending_scalar = (out_ext[rows, c0 + hw:c0 + wu],
                                          ot[:, OM + hw:OM + wu])
            if pending_scalar is not None:
                nc.scalar.dma_start(*pending_scalar)
    nc.compile()
    return nc


_GRAPH_CACHE = {}


def _get_graph():
    if "nc" not in _GRAPH_CACHE:
        _GRAPH_CACHE["nc"] = build_graph()
    return _GRAPH_CACHE["nc"]


def kernel(key_map: np.ndarray, query_map: np.ndarray,
           _trace: bool = False, _tmpdir: str | None = None):
    import ml_dtypes
    bf16 = ml_dtypes.bfloat16
    key_map = np.ascontiguousarray(key_map, dtype=np.float32).astype(bf16)
    query_map = np.ascontiguousarray(query_map, dtype=np.float32).astype(bf16)
    assert key_map.shape == (B, C, H, W), key_map.shape

    nc = _get_graph()
    in_maps = [
        {"key_map": key_map[b].reshape(C, L),
         "query_map": query_map[b].reshape(C, L)}
        for b in range(B)
    ]
    res = run_bass_kernel_spmd(
        nc, in_maps, core_ids=list(range(B)),
        trace=_trace, tmpdir=_tmpdir,
    )
    out = np.stack([res.results[b]["out"] for b in range(B)])
    _GRAPH_CACHE["last_exec_time_ns"] = res.exec_time_ns
    _GRAPH_CACHE["last_results"] = res
    return out.astype(np.float32).reshape(B, C, L, K2)
